# revision 1
# baseline (speedup 1.0000x reference)
"""Self-contained 8-core Trainium2 Bass kernel for nn_MultiHeadAttention.

Full (unsharded) inputs in, full output out. Sharding: core c handles
batch b = c // 2, query-half h = c % 2 (1024 queries). K/V projections for a
batch are computed redundantly on the 2 cores sharing it -> zero collectives,
disjoint outputs.

All matmuls run in float32r (TF32-like, ~1.6e-4 rel err, 4x fp32 throughput).
"""

import ml_dtypes
import numpy as np

import concourse.bass as bass
import concourse.mybir as mybir
from concourse import bacc
from concourse.tile import TileContext
from concourse.bass_utils import run_bass_kernel_spmd

F32 = mybir.dt.float32
F32R = mybir.dt.float32r
BF16 = mybir.dt.bfloat16
ACT = mybir.ActivationFunctionType

B, S, D = 4, 2048, 1024
H, DK = 16, 64
SQ = S // 2            # queries per core
P = 128
NCORES = 8
FC = D // P            # 8 feature chunks (contraction)
OFC = D // P           # 8 output-feature chunks
PAIRS = H // 2         # 8 head pairs (2 heads = 128 partitions)
NKT = S // P           # 16 key tiles of 128 tokens
QTW = 512              # q tile width
NQT = SQ // QTW        # 2
EB = 2                 # key tiles per exp batch (2 psum banks per scores tile)
SCALE = 1.0 / np.sqrt(np.float32(DK))


def build_nc():
    nc = bacc.Bacc()

    xq = nc.declare_dram_parameter("xq_t", [D, SQ], F32R, isOutput=False)
    xk = nc.declare_dram_parameter("xk_t", [D, S], F32R, isOutput=False)
    xv = nc.declare_dram_parameter("xv_t", [D, S], F32R, isOutput=False)
    wq = nc.declare_dram_parameter("wq_t", [D, D], F32R, isOutput=False)
    wk = nc.declare_dram_parameter("wk_t", [D, D], F32R, isOutput=False)
    wv = nc.declare_dram_parameter("wv_t", [D, D], F32R, isOutput=False)
    wo = nc.declare_dram_parameter("wo_t", [D, D], BF16, isOutput=False)
    bq = nc.declare_dram_parameter("b_q_r", [P, OFC], F32, isOutput=False)
    bk = nc.declare_dram_parameter("b_k_r", [P, OFC], F32, isOutput=False)
    bo = nc.declare_dram_parameter("b_o_r", [P, OFC], F32, isOutput=False)
    ones_in = nc.declare_dram_parameter("ones_row", [1, P], F32R, isOutput=False)
    vones_in = nc.declare_dram_parameter("v_ones", [P, NKT, 2, 1], F32R,
                                         isOutput=False)
    out = nc.declare_dram_parameter("out_t", [D, SQ], F32, isOutput=True)

    kt_scr = nc.dram_tensor("kt_scr", [P, PAIRS, S], F32R)
    v_scr = nc.dram_tensor("v_scr", [P, PAIRS, NKT, 2, DK], F32R)

    with nc.allow_low_precision(reason="f32r compute"), TileContext(nc) as tc:
        with (
            tc.tile_pool(name="persist", bufs=1) as pers,
            tc.tile_pool(name="norm", bufs=2) as npool,
        ):
            qt_s = pers.tile([P, OFC, SQ], F32R, tag="qt")
            attn_t = pers.tile([P, PAIRS, SQ], BF16, tag="attnt")
            tbq = pers.tile([P, OFC], F32, tag="tbq")
            tbk = pers.tile([P, OFC], F32, tag="tbk")
            tbo = pers.tile([P, OFC], F32, tag="tbo")
            tones = pers.tile([1, P], F32R, tag="tones")
            vones_s = pers.tile([P, NKT, 2, 1], F32R, tag="vones")
            nc.sync.dma_start(tbq[:], bq[:])
            nc.sync.dma_start(tbk[:], bk[:])
            nc.sync.dma_start(tbo[:], bo[:])
            nc.sync.dma_start(tones[:], ones_in[:])
            nc.sync.dma_start(vones_s[:], vones_in[:])

            # ---------------- Stage A: projections ----------------
            # kpairpool is opened alongside stage A so the first head-pair's
            # K/V loads can overlap the stage-A tail.
            kpp_cm = tc.tile_pool(name="kpairpool", bufs=2)
            kpp = kpp_cm.__enter__()
            with (
                tc.tile_pool(name="wpool", bufs=2) as wpool,
                tc.tile_pool(name="xpool", bufs=2) as xpool,
                tc.tile_pool(name="kspool", bufs=3) as kspool,
                tc.tile_pool(name="apsum", bufs=4, space="PSUM") as apsum,
            ):
                def load_w(src, dt=F32R):
                    wt = wpool.tile([P, FC, D], dt, tag="w_full")
                    for fc in range(FC):
                        nc.sync.dma_start(wt[:, fc, :],
                                          src[fc * P:(fc + 1) * P, :])
                    return wt

                def load_x(src, tt, dt=F32R):
                    xts = []
                    for fc in range(FC):
                        xt = xpool.tile([P, QTW], dt, tag=f"x{fc}")
                        nc.sync.dma_start(
                            xt[:], src[fc * P:(fc + 1) * P,
                                       tt * QTW:(tt + 1) * QTW])
                        xts.append(xt)
                    return xts

                # --- Q projection: QT[of, t] = Wq @ xq ---
                # Interleave W and x DMAs, and split each W row-chunk so the
                # first matmul's 128 columns arrive in ~1us, not after the
                # whole W load.
                wtq = wpool.tile([P, FC, D], F32R, tag="w_full")
                xq_tiles = [[], []]
                for fc in range(FC):
                    nc.sync.dma_start(wtq[:, fc, 0:P],
                                      wq[fc * P:(fc + 1) * P, 0:P])
                    for tt in range(NQT):
                        xt = xpool.tile([P, QTW], F32R, tag=f"x{fc}",
                                        name=f"xq{fc}_{tt}")
                        nc.sync.dma_start(
                            xt[:], xq[fc * P:(fc + 1) * P,
                                      tt * QTW:(tt + 1) * QTW])
                        xq_tiles[tt].append(xt)
                    nc.sync.dma_start(wtq[:, fc, P:],
                                      wq[fc * P:(fc + 1) * P, P:])
                for tt in range(NQT):
                    xts = xq_tiles[tt]
                    for ofc in range(OFC):
                        ps = apsum.tile([P, QTW], F32, tag="acc")
                        for fc in range(FC):
                            nc.tensor.matmul(
                                ps[:],
                                wtq[:, fc, ofc * P:(ofc + 1) * P],
                                xts[fc][:],
                                start=(fc == 0), stop=(fc == FC - 1),
                            )
                        nc.vector.tensor_scalar_add(
                            qt_s[:, ofc, tt * QTW:(tt + 1) * QTW], ps[:],
                            tbq[:, ofc:ofc + 1])

                # --- K projection -> kt_scr[of, t] (DRAM bounce) ---
                wtk = load_w(wk)
                for tt in range(S // QTW):
                    xts = load_x(xk, tt)
                    for ofc in range(OFC):
                        ps = apsum.tile([P, QTW], F32, tag="acc")
                        for fc in range(FC):
                            nc.tensor.matmul(
                                ps[:],
                                wtk[:, fc, ofc * P:(ofc + 1) * P],
                                xts[fc][:],
                                start=(fc == 0), stop=(fc == FC - 1),
                            )
                        ks = kspool.tile([P, QTW], F32R, tag="ks")
                        nc.vector.tensor_scalar_add(ks[:], ps[:],
                                                    tbk[:, ofc:ofc + 1])
                        nc.sync.dma_start(
                            kt_scr[:, ofc, tt * QTW:(tt + 1) * QTW], ks[:])

                # --- V projection: V[t, of] (natural), no bias (folded) ---
                wtv = load_w(wv)
                for ktg in range(4):  # groups of 4 key tiles (512 tokens)
                    xts = load_x(xv, ktg)
                    for ki in range(4):
                        kt = ktg * 4 + ki
                        for half in range(2):
                            ps = apsum.tile([P, QTW], F32, tag="acc")
                            for fc in range(FC):
                                nc.tensor.matmul(
                                    ps[:],
                                    xts[fc][:, ki * P:(ki + 1) * P],
                                    wtv[:, fc, half * 512:(half + 1) * 512],
                                    start=(fc == 0), stop=(fc == FC - 1),
                                )
                            # scatter 512 of-cols (= 4 pairs x 2 heads x 64)
                            # to DRAM scratch via SBUF staging
                            vs = kspool.tile([P, QTW], F32R, tag="vs")
                            nc.vector.tensor_copy(vs[:], ps[:])
                            nc.sync.dma_start(
                                v_scr[:, half * 4:(half + 1) * 4, kt, :, :],
                                vs[:].rearrange(
                                    "p (c h2 d) -> p c h2 d", c=4, h2=2),
                            )

            # ---------------- Stage B: attention, head-pairs row-tiled ----
            with tc.tile_pool(name="wopool", bufs=1) as wop:
                # prefetch W_out (bf16) for stage C while attention runs
                wto = wop.tile([P, FC, D], BF16, tag="wo_full")
                for fc in range(FC):
                    nc.sync.dma_start(wto[:, fc, :], wo[fc * P:(fc + 1) * P, :])

                with (
                    tc.tile_pool(name="ppool", bufs=2) as ppool,
                    tc.tile_pool(name="utpool", bufs=2) as utp,
                    tc.tile_pool(name="bpsum", bufs=1, space="PSUM") as bpsum,
                ):
                    self_attention(nc, tc, kpp, ppool, utp, bpsum, npool,
                                   kt_scr, v_scr, qt_s, attn_t, vones_s, tones)

                # ---------- Stage C: out projection (transposed) ----------
                with (
                    tc.tile_pool(name="opool", bufs=3) as opool,
                    tc.tile_pool(name="cpsum", bufs=3, space="PSUM") as cpsum,
                ):
                    for tt in range(NQT):
                        for ofc in range(OFC):
                            ps = cpsum.tile([P, QTW], F32, tag="oacc")
                            for c in range(PAIRS):
                                nc.tensor.matmul(
                                    ps[:],
                                    wto[:, c, ofc * P:(ofc + 1) * P],
                                    attn_t[:, c, tt * QTW:(tt + 1) * QTW],
                                    start=(c == 0), stop=(c == PAIRS - 1),
                                )
                            osb = opool.tile([P, QTW], F32, tag="osb")
                            nc.vector.tensor_scalar_add(osb[:], ps[:],
                                                        tbo[:, ofc:ofc + 1])
                            nc.sync.dma_start(
                                out[ofc * P:(ofc + 1) * P,
                                    tt * QTW:(tt + 1) * QTW], osb[:])

            kpp_cm.__exit__(None, None, None)

    nc.finalize()
    return nc


def self_attention(nc, tc, kpp, ppool, utp, bpsum, npool, kt_scr, v_scr,
       qt_s, attn_t, vones_s, tones):
    for c in range(PAIRS):
        kpair = kpp.tile([P, S], F32R, tag="kpair")
        nc.sync.dma_start(kpair[:], kt_scr[:, c, :])
        vpair = kpp.tile([P, NKT, 2, DK + 1], F32R, tag="vpair")
        nc.sync.dma_start(vpair[:, :, :, 0:DK], v_scr[:, c])
        nc.vector.tensor_copy(vpair[:, :, :, DK:DK + 1], vones_s[:])
        for qt in range(NQT):
            qsl = slice(qt * QTW, (qt + 1) * QTW)
            # both heads of the pair: scores via row-tiled
            # concurrent K=64 matmuls (rows 0-63 / 64-127)
            # P-tiles split in two kt-halves for finer exp/PV pipelining
            pts = [[ppool.tile([P, NKT // 2, QTW], F32R,
                               tag=f"pt{h2}{hf}", name=f"pt{h2}{hf}", bufs=1)
                    for hf in range(2)]
                   for h2 in range(2)]
            for ktb in range(NKT // EB):
                sps = [bpsum.tile([P, EB * QTW], F32,
                                  tag=f"sc{h2}", name=f"sc{h2}")
                       for h2 in range(2)]
                for e in range(EB):
                    kt = ktb * EB + e
                    for h2 in range(2):
                        base = h2 * DK
                        nc.tensor.matmul(
                            sps[h2][:, e * QTW:(e + 1) * QTW],
                            kpair[base:base + DK,
                                  kt * P:(kt + 1) * P],
                            qt_s[base:base + DK, c, qsl],
                            start=True, stop=True,
                            tile_position=(base, 0),
                        )
                for h2 in range(2):
                    kb0 = ktb * EB
                    hf, off = divmod(kb0, NKT // 2)
                    nc.scalar.activation(
                        pts[h2][hf][:, off:off + EB, :],
                        sps[h2][:], ACT.Exp, scale=float(SCALE))
            for h2 in range(2):
                base = h2 * DK
                ut = bpsum.tile([DK + 1, QTW], F32, tag="ut", bufs=2)
                for kt in range(NKT):
                    hf, koff = divmod(kt, NKT // 2)
                    nc.tensor.matmul(
                        ut[:],
                        vpair[:, kt, h2, :],
                        pts[h2][hf][:, koff, :],
                        start=(kt == 0), stop=(kt == NKT - 1),
                    )
                recip = npool.tile([1, QTW], F32R, tag="recip")
                nc.vector.reciprocal(recip[:], ut[DK:DK + 1, :])
                bc = bpsum.tile([P, QTW], F32, tag="bc", bufs=2)
                nc.tensor.matmul(bc[:], tones[:], recip[:],
                                 start=True, stop=True)
                uts = utp.tile([DK, QTW], F32, tag="uts")
                nc.vector.tensor_copy(uts[:], ut[0:DK, :])
                nc.vector.tensor_mul(
                    attn_t[base:base + DK, c, qsl],
                    uts[:], bc[0:DK, :])
    return nc


def _prep_host(query, key, value, W_q, b_q, W_k, b_k, W_v, b_v, W_out, b_out):
    """Host-side layout prep (transposes / bias folding). No math beyond the
    b_v fold, which is a 1024x1024 matvec."""
    f32 = np.float32
    query = np.asarray(query, f32)
    key = np.asarray(key, f32)
    value = np.asarray(value, f32)
    W_q = np.asarray(W_q, f32)
    W_k = np.asarray(W_k, f32)
    W_v = np.asarray(W_v, f32)
    W_out = np.asarray(W_out, f32)
    b_q = np.asarray(b_q, f32)
    b_k = np.asarray(b_k, f32)
    b_v = np.asarray(b_v, f32)
    b_out = np.asarray(b_out, f32)

    common = {
        "wq_t": np.ascontiguousarray(W_q.T),
        "wk_t": np.ascontiguousarray(W_k.T),
        "wv_t": np.ascontiguousarray(W_v.T),
        "wo_t": np.ascontiguousarray(W_out.T).astype(ml_dtypes.bfloat16),
        "b_q_r": np.ascontiguousarray(b_q.reshape(OFC, P).T),
        "b_k_r": np.ascontiguousarray(b_k.reshape(OFC, P).T),
        "b_o_r": np.ascontiguousarray(
            (b_out + W_out @ b_v).reshape(OFC, P).T.astype(f32)),
        "ones_row": np.ones((1, P), f32),
        "v_ones": np.ones((P, NKT, 2, 1), f32),
    }
    in_maps = []
    for c in range(NCORES):
        b, hf = divmod(c, 2)
        m = dict(common)
        m["xq_t"] = np.ascontiguousarray(
            query[b, hf * SQ:(hf + 1) * SQ, :].T)
        m["xk_t"] = np.ascontiguousarray(key[b].T)
        m["xv_t"] = np.ascontiguousarray(value[b].T)
        in_maps.append(m)
    return in_maps


_NC_CACHE = {}


def get_nc():
    if "nc" not in _NC_CACHE:
        _NC_CACHE["nc"] = build_nc()
    return _NC_CACHE["nc"]


def get_runner():
    """Build (once) a cached jitted SPMD callable over 8 cores.

    Mirrors concourse.bass2jax.run_bass_via_pjrt's multi-core path, but keeps
    the jitted function so repeated calls don't recompile the NEFF.
    """
    if "runner" in _NC_CACHE:
        return _NC_CACHE["runner"]

    import jax
    from jax.experimental.shard_map import shard_map
    from jax.sharding import Mesh, PartitionSpec

    from concourse import bass2jax

    nc = get_nc()
    bass2jax.install_neuronx_cc_hook()
    partition_name = (
        nc.partition_id_tensor.name if nc.partition_id_tensor else None
    )

    in_names, out_names, out_avals, zero_shapes = [], [], [], []
    for alloc in nc.m.functions[0].allocations:
        if not isinstance(alloc, mybir.MemoryLocationSet):
            continue
        name = alloc.memorylocations[0].name
        if alloc.kind == "ExternalInput":
            if name != partition_name:
                in_names.append(name)
        elif alloc.kind == "ExternalOutput":
            shape = tuple(alloc.tensor_shape)
            dtype = mybir.dt.np(alloc.dtype)
            out_names.append(name)
            out_avals.append(jax.core.ShapedArray(shape, dtype))
            zero_shapes.append((shape, dtype))
    n_params = len(in_names)
    n_outs = len(out_names)
    all_names = in_names + out_names
    if partition_name is not None:
        all_names = all_names + [partition_name]
    donate = tuple(range(n_params, n_params + n_outs))

    def _body(*args):
        operands = list(args)
        if partition_name is not None:
            operands.append(bass2jax.partition_id_tensor())
        outs = bass2jax._bass_exec_p.bind(
            *operands,
            out_avals=tuple(out_avals),
            in_names=tuple(all_names),
            out_names=tuple(out_names),
            lowering_input_output_aliases=(),
            sim_require_finite=True,
            sim_require_nnan=True,
            nc=nc,
        )
        return tuple(outs)

    devices = jax.devices()[:NCORES]
    mesh = Mesh(np.asarray(devices), ("core",))
    in_specs = (PartitionSpec("core"),) * (n_params + n_outs)
    out_specs = (PartitionSpec("core"),) * n_outs
    sharded = jax.jit(
        shard_map(_body, mesh=mesh, in_specs=in_specs, out_specs=out_specs,
                  check_rep=False),
        donate_argnums=donate,
        keep_unused=True,
    )

    def run(in_maps):
        concat_in = [
            np.concatenate([np.asarray(in_maps[c][n]) for c in range(NCORES)],
                           axis=0)
            for n in in_names
        ]
        zeros = [np.zeros((NCORES * s[0], *s[1:]), d) for s, d in zero_shapes]
        out_arrs = sharded(*concat_in, *zeros)
        return [
            {
                n: np.asarray(out_arrs[i]).reshape(
                    NCORES, *out_avals[i].shape)[c]
                for i, n in enumerate(out_names)
            }
            for c in range(NCORES)
        ]

    runner = {
        "run": run,
        "sharded": sharded,
        "in_names": in_names,
        "out_names": out_names,
        "out_avals": out_avals,
        "zero_shapes": zero_shapes,
        "mesh": mesh,
    }
    _NC_CACHE["runner"] = runner
    return runner


def kernel(**inputs) -> np.ndarray:
    in_maps = _prep_host(**inputs)
    results = get_runner()["run"](in_maps)
    out = np.empty((B, S, D), np.float32)
    for c in range(NCORES):
        b, hf = divmod(c, 2)
        out[b, hf * SQ:(hf + 1) * SQ, :] = results[c]["out_t"].T
    return out



# revision 10
# speedup vs baseline: 1.2082x; 1.2082x over previous
"""Self-contained 8-core Trainium2 Bass kernel for nn_MultiHeadAttention.

Full (unsharded) inputs in, full output out. Sharding: core c handles
batch b = c // 2, query-half h = c % 2 (1024 queries). K/V projections for a
batch are computed redundantly on the 2 cores sharing it -> zero collectives,
disjoint outputs.

v3: bf16 compute, SBUF-resident K/V/Q (no DRAM bounce), K projection per
head-pair, PV in transposed orientation (probs stationary, [V|1] moving,
65-wide) with per-partition softmax normalization, and a software pipeline
that keeps scores one block ahead of PV so the scalar-engine exp stream
never starves and never throttles the PE.
"""

import ml_dtypes
import numpy as np

import concourse.bass as bass
import concourse.mybir as mybir
from concourse import bacc
from concourse.tile import TileContext
from concourse.bass_utils import run_bass_kernel_spmd

F32 = mybir.dt.float32
F32R = mybir.dt.float32r
BF16 = mybir.dt.bfloat16
ACT = mybir.ActivationFunctionType

B, S, D = 4, 2048, 1024
H, DK = 16, 64
SQ = S // 2            # queries per core
P = 128
NCORES = 8
FC = D // P            # 8 feature chunks (contraction)
PAIRS = H // 2         # 8 head pairs (2 heads = 128 partitions)
NKT = S // P           # 16 key tiles of 128 tokens
QTW = 512              # q tile width
NQT = SQ // QTW        # 2
EB = 2                 # key tiles per scores psum tile (per head)
SCALE = 1.0 / np.sqrt(np.float32(DK))


def build_nc():
    nc = bacc.Bacc()

    xq = nc.declare_dram_parameter("xq_bf", [D, SQ], BF16, isOutput=False)
    xk = nc.declare_dram_parameter("xk_bf", [D, S], BF16, isOutput=False)
    xv = nc.declare_dram_parameter("xv_bf", [D, S], BF16, isOutput=False)
    wq = nc.declare_dram_parameter("wq_bf", [D, D], BF16, isOutput=False)
    wk = nc.declare_dram_parameter("wk_bf", [D, D], BF16, isOutput=False)
    wv = nc.declare_dram_parameter("wv_bf", [D, D], BF16, isOutput=False)
    wo = nc.declare_dram_parameter("wo_bf", [D, D], BF16, isOutput=False)
    bq = nc.declare_dram_parameter("b_q_r", [P, FC], F32, isOutput=False)
    bk = nc.declare_dram_parameter("b_k_r", [P, FC], F32, isOutput=False)
    bo = nc.declare_dram_parameter("b_o_r", [P, FC], F32, isOutput=False)
    ident = nc.declare_dram_parameter("ident_bf", [P, P], BF16, isOutput=False)
    out = nc.declare_dram_parameter("out_t", [D, SQ], F32, isOutput=True)

    with nc.allow_low_precision(reason="bf16 compute"), TileContext(nc) as tc:
        with (
            tc.tile_pool(name="persist", bufs=1) as pers,
            tc.tile_pool(name="npool", bufs=2) as npool,
        ):
            # ---- persistent SBUF tiles ----
            qt_s = pers.tile([P, FC, SQ], BF16, tag="qt")        # Q^T, all pairs
            attn_t = pers.tile([P, PAIRS, SQ], BF16, tag="attnt")
            wv_s = pers.tile([P, FC, D], BF16, tag="wvs")        # full W_v^T
            tbq = pers.tile([P, FC], F32, tag="tbq")
            tbk = pers.tile([P, FC], F32, tag="tbk")
            tbo = pers.tile([P, FC], F32, tag="tbo")
            tid = pers.tile([P, P], BF16, tag="tid")

            nc.sync.dma_start(tbq[:], bq[:])
            nc.sync.dma_start(tbk[:], bk[:])
            nc.sync.dma_start(tbo[:], bo[:])
            nc.sync.dma_start(tid[:], ident[:])

            with (
                tc.tile_pool(name="kpool", bufs=2) as kp,
                tc.tile_pool(name="kwpool", bufs=2) as kwp,
            ):
                kpairs = {}

                def k_proj_pair(c, psum):
                    # stream this pair's W_k columns (8 x [128,128] bf16)
                    wkc = []
                    for fc in range(FC):
                        wt = kwp.tile([P, P], BF16, tag=f"wk{fc}",
                                      name=f"wk{fc}_{c}")
                        nc.sync.dma_start(
                            wt[:], wk[fc * P:(fc + 1) * P,
                                      c * P:(c + 1) * P])
                        wkc.append(wt)
                    k_pair = kp.tile([P, S], BF16, tag="kp", name=f"kp{c}")
                    for tt in range(S // QTW):
                        ps = psum.tile([P, QTW], F32, tag="acc", name="psk")
                        for fc in range(FC):
                            nc.tensor.matmul(
                                ps[:],
                                wkc[fc][:],
                                xk_s[:, fc, tt * QTW:(tt + 1) * QTW],
                                start=(fc == 0), stop=(fc == FC - 1),
                            )
                        nc.vector.tensor_scalar_add(
                            k_pair[:, tt * QTW:(tt + 1) * QTW], ps[:],
                            tbk[:, c:c + 1])
                    kpairs[c] = k_pair

                # ---- Q projection + K(pair 0), own psum scope ----
                with (
                    tc.tile_pool(name="qpool", bufs=1) as qpool,
                    tc.tile_pool(name="qpsum", bufs=6, space="PSUM") as qpsum,
                ):
                    wq_t, xq_t = [], []
                    for fc in range(FC):
                        wt = qpool.tile([P, D], BF16, tag=f"wq{fc}",
                                        name=f"wq{fc}")
                        nc.sync.dma_start(wt[:], wq[fc * P:(fc + 1) * P, :])
                        wq_t.append(wt)
                        xt = qpool.tile([P, SQ], BF16, tag=f"xq{fc}",
                                        name=f"xq{fc}")
                        nc.sync.dma_start(xt[:], xq[fc * P:(fc + 1) * P, :])
                        xq_t.append(xt)
                    # K resident loads (needed at ~30us)
                    for fc in range(FC):
                        nc.sync.dma_start(xk_s[:, fc, :],
                                          xk[fc * P:(fc + 1) * P, :])
                    for c in range(PAIRS):
                        for tt in range(NQT):
                            ps = qpsum.tile([P, QTW], F32, tag="acc",
                                            name="psq")
                            for fc in range(FC):
                                nc.tensor.matmul(
                                    ps[:],
                                    wq_t[fc][:, c * P:(c + 1) * P],
                                    xq_t[fc][:, tt * QTW:(tt + 1) * QTW],
                                    start=(fc == 0), stop=(fc == FC - 1),
                                )
                            nc.vector.tensor_scalar_add(
                                qt_s[:, c, tt * QTW:(tt + 1) * QTW], ps[:],
                                tbq[:, c:c + 1])
                    k_proj_pair(0, qpsum)

                # ---- main pipeline scopes ----
                with (
                    tc.tile_pool(name="ppool", bufs=2) as ppool,
                    tc.tile_pool(name="spsum", bufs=1, space="PSUM") as spsum,
                    tc.tile_pool(name="apsum", bufs=3, space="PSUM") as apsum,
                    tc.tile_pool(name="tpsum", bufs=1, space="PSUM") as tpsum,
                ):
                    def scores(c, qt):
                        """scores + exp for block (c, qt) -> pts tile.

                        Two alternating 2-bank sps tags so the next pair of
                        score matmuls runs while the previous exp drains."""
                        qsl = slice(qt * QTW, (qt + 1) * QTW)
                        k_pair = kpairs[c]
                        pts = ppool.tile([P, 2, NKT, QTW], BF16, tag="pts",
                                         name=f"pts{c}_{qt}")
                        for sg in range(NKT * 2 // EB):
                            h2, kb = divmod(sg, NKT // EB)
                            base = h2 * DK
                            sps = spsum.tile([P, EB, QTW], F32,
                                             tag=f"sps{sg % 2}",
                                             name=f"sps{sg % 2}")
                            for e in range(EB):
                                kt = kb * EB + e
                                nc.tensor.matmul(
                                    sps[:, e, :],
                                    k_pair[base:base + DK,
                                           kt * P:(kt + 1) * P],
                                    qt_s[base:base + DK, c, qsl],
                                    start=True, stop=True,
                                    tile_position=(base, 0),
                                )
                            nc.scalar.activation(
                                pts[:, h2, kb * EB:(kb + 1) * EB, :],
                                sps[:], ACT.Exp, scale=float(SCALE))
                        return pts

                    def pv_block(c, qt, pts, v_all):
                        """PV in transposed orientation + normalize +
                        transpose back to head-major attn_t."""
                        qsl = slice(qt * QTW, (qt + 1) * QTW)
                        att_sb = npool.tile([P, 4, P], BF16, tag="attsb",
                                            name="attsb")
                        for h2 in range(2):
                            head = 2 * c + h2
                            for qc in range(4):
                                ut = apsum.tile([P, QTW], F32, tag="acc",
                                                name="ut")
                                for kt in range(NKT):
                                    nc.tensor.matmul(
                                        ut[:, 0:DK + 1],
                                        pts[:, h2, kt,
                                            qc * P:(qc + 1) * P],
                                        v_all[:, kt, head, :],
                                        start=(kt == 0), stop=(kt == NKT - 1),
                                    )
                                recip = npool.tile([P, 1], F32, tag="recip",
                                                   name="recip")
                                nc.vector.reciprocal(recip[:],
                                                     ut[:, DK:DK + 1])
                                nc.vector.tensor_scalar_mul(
                                    att_sb[:, qc, h2 * DK:(h2 + 1) * DK],
                                    ut[:, 0:DK], recip[:])
                        pt_ps = tpsum.tile([P, QTW], BF16, tag="ptp",
                                           name="ptp")
                        for qc in range(4):
                            nc.tensor.transpose(
                                pt_ps[:, qc * P:(qc + 1) * P],
                                att_sb[:, qc, :], tid[:])
                        nc.vector.tensor_copy(attn_t[:, c, qsl], pt_ps[:])

                    def v_proj_half(half, v_all, wv_s, xvp):
                        # heads 8*half..8*half+7 ; natural layout, no bias
                        # (b_v folded into b_out on host)
                        for ktg in range(4):
                            xvt = []
                            for fc in range(FC):
                                xt = xvp.tile([P, QTW], BF16, tag=f"xv{fc}",
                                              name=f"xv{fc}_{half}_{ktg}")
                                nc.sync.dma_start(
                                    xt[:], xv[fc * P:(fc + 1) * P,
                                              ktg * QTW:(ktg + 1) * QTW])
                                xvt.append(xt)
                            for ki in range(4):
                                kt = ktg * 4 + ki
                                ps = apsum.tile([P, QTW], F32, tag="acc",
                                                name="psv")
                                for fc in range(FC):
                                    nc.tensor.matmul(
                                        ps[:],
                                        xvt[fc][:, ki * P:(ki + 1) * P],
                                        wv_s[:, fc,
                                             half * QTW:(half + 1) * QTW],
                                        start=(fc == 0), stop=(fc == FC - 1),
                                    )
                                nc.vector.tensor_copy(
                                    v_all[:, kt, half * 8:(half + 1) * 8,
                                          0:DK],
                                    ps[:].rearrange("p (h d) -> p h d", h=8))

                    def run_blocks(blocks, pts_q, v_all):
                        for b in blocks:
                            c, qt = b
                            nb = (c, qt + 1) if qt + 1 < NQT else (c + 1, 0)
                            if nb[0] < PAIRS:
                                pts_q[nb] = scores(*nb)
                            pv_block(c, qt, pts_q.pop(b), v_all)
                            if qt == 0 and c + 1 < PAIRS:
                                k_proj_pair(c + 1, apsum)

                    # ---- pipelined attention ----
                    blocks = [(c, qt) for c in range(PAIRS)
                              for qt in range(NQT)]
                    with tc.tile_pool(name="vpool", bufs=1) as vpool:
                        v_all = vpool.tile([P, NKT, H, DK + 1], BF16,
                                           tag="vall")
                        nc.vector.memset(v_all[:, :, :, DK:DK + 1], 1.0)
                        pts_q = {}

                        with (
                            tc.tile_pool(name="wvpool", bufs=1) as wvp,
                            tc.tile_pool(name="xvpool", bufs=1) as xvp,
                        ):
                            wv_s = wvp.tile([P, FC, D], BF16, tag="wvs")
                            for fc in range(FC):
                                nc.sync.dma_start(wv_s[:, fc, :],
                                                  wv[fc * P:(fc + 1) * P, :])

                            pts_q[blocks[0]] = scores(*blocks[0])
                            v_proj_half(0, v_all, wv_s, xvp)
                            run_blocks(blocks[:8], pts_q, v_all)
                            v_proj_half(1, v_all, wv_s, xvp)

                        # ---- second half + out projection ----
                        with tc.tile_pool(name="wopool", bufs=1) as wop:
                            wo_s = wop.tile([P, FC, D], BF16, tag="wos")
                            for fc in range(FC):
                                nc.sync.dma_start(wo_s[:, fc, :],
                                                  wo[fc * P:(fc + 1) * P, :])

                            run_blocks(blocks[8:], pts_q, v_all)

                            for tt in range(NQT):
                                for ofc in range(FC):
                                    ps = apsum.tile([P, QTW], F32, tag="acc",
                                                    name="pso")
                                    for c in range(PAIRS):
                                        nc.tensor.matmul(
                                            ps[:],
                                            wo_s[:, c, ofc * P:(ofc + 1) * P],
                                            attn_t[:, c,
                                                   tt * QTW:(tt + 1) * QTW],
                                            start=(c == 0),
                                            stop=(c == PAIRS - 1),
                                        )
                                    osb = npool.tile([P, QTW], F32, tag="osb",
                                                     name="osb")
                                    nc.vector.tensor_scalar_add(
                                        osb[:], ps[:], tbo[:, ofc:ofc + 1])
                                    nc.sync.dma_start(
                                        out[ofc * P:(ofc + 1) * P,
                                            tt * QTW:(tt + 1) * QTW], osb[:])

    nc.finalize()
    return nc


def _prep_host(query, key, value, W_q, b_q, W_k, b_k, W_v, b_v, W_out, b_out):
    """Host-side layout prep (transposes / bias folding / bf16 casts). No math
    beyond the b_v fold, which is a 1024x1024 matvec."""
    f32 = np.float32
    bf16 = ml_dtypes.bfloat16
    query = np.asarray(query, f32)
    key = np.asarray(key, f32)
    value = np.asarray(value, f32)
    W_q = np.asarray(W_q, f32)
    W_k = np.asarray(W_k, f32)
    W_v = np.asarray(W_v, f32)
    W_out = np.asarray(W_out, f32)
    b_q = np.asarray(b_q, f32)
    b_k = np.asarray(b_k, f32)
    b_v = np.asarray(b_v, f32)
    b_out = np.asarray(b_out, f32)

    common = {
        "wq_bf": np.ascontiguousarray(W_q.T).astype(bf16),
        "wk_bf": np.ascontiguousarray(W_k.T).astype(bf16),
        "wv_bf": np.ascontiguousarray(W_v.T).astype(bf16),
        "wo_bf": np.ascontiguousarray(W_out.T).astype(bf16),
        "b_q_r": np.ascontiguousarray(b_q.reshape(FC, P).T),
        "b_k_r": np.ascontiguousarray(b_k.reshape(FC, P).T),
        "b_o_r": np.ascontiguousarray(
            (b_out + W_out @ b_v).reshape(FC, P).T.astype(f32)),
        "ident_bf": np.eye(P, dtype=bf16),
    }
    in_maps = []
    for c in range(NCORES):
        b, hf = divmod(c, 2)
        m = dict(common)
        m["xq_bf"] = np.ascontiguousarray(
            query[b, hf * SQ:(hf + 1) * SQ, :].T).astype(bf16)
        m["xk_bf"] = np.ascontiguousarray(key[b].T).astype(bf16)
        m["xv_bf"] = np.ascontiguousarray(value[b].T).astype(bf16)
        in_maps.append(m)
    return in_maps


_NC_CACHE = {}


def get_nc():
    if "nc" not in _NC_CACHE:
        _NC_CACHE["nc"] = build_nc()
    return _NC_CACHE["nc"]


def get_runner():
    """Build (once) a cached jitted SPMD callable over 8 cores.

    Mirrors concourse.bass2jax.run_bass_via_pjrt's multi-core path, but keeps
    the jitted function so repeated calls don't recompile the NEFF.
    """
    if "runner" in _NC_CACHE:
        return _NC_CACHE["runner"]

    import jax
    from jax.experimental.shard_map import shard_map
    from jax.sharding import Mesh, PartitionSpec

    from concourse import bass2jax

    nc = get_nc()
    bass2jax.install_neuronx_cc_hook()
    partition_name = (
        nc.partition_id_tensor.name if nc.partition_id_tensor else None
    )

    in_names, out_names, out_avals, zero_shapes = [], [], [], []
    for alloc in nc.m.functions[0].allocations:
        if not isinstance(alloc, mybir.MemoryLocationSet):
            continue
        name = alloc.memorylocations[0].name
        if alloc.kind == "ExternalInput":
            if name != partition_name:
                in_names.append(name)
        elif alloc.kind == "ExternalOutput":
            shape = tuple(alloc.tensor_shape)
            dtype = mybir.dt.np(alloc.dtype)
            out_names.append(name)
            out_avals.append(jax.core.ShapedArray(shape, dtype))
            zero_shapes.append((shape, dtype))
    n_params = len(in_names)
    n_outs = len(out_names)
    all_names = in_names + out_names
    if partition_name is not None:
        all_names = all_names + [partition_name]
    donate = tuple(range(n_params, n_params + n_outs))

    def _body(*args):
        operands = list(args)
        if partition_name is not None:
            operands.append(bass2jax.partition_id_tensor())
        outs = bass2jax._bass_exec_p.bind(
            *operands,
            out_avals=tuple(out_avals),
            in_names=tuple(all_names),
            out_names=tuple(out_names),
            lowering_input_output_aliases=(),
            sim_require_finite=True,
            sim_require_nnan=True,
            nc=nc,
        )
        return tuple(outs)

    devices = jax.devices()[:NCORES]
    mesh = Mesh(np.asarray(devices), ("core",))
    in_specs = (PartitionSpec("core"),) * (n_params + n_outs)
    out_specs = (PartitionSpec("core"),) * n_outs
    sharded = jax.jit(
        shard_map(_body, mesh=mesh, in_specs=in_specs, out_specs=out_specs,
                  check_rep=False),
        donate_argnums=donate,
        keep_unused=True,
    )

    def run(in_maps):
        concat_in = [
            np.concatenate([np.asarray(in_maps[c][n]) for c in range(NCORES)],
                           axis=0)
            for n in in_names
        ]
        zeros = [np.zeros((NCORES * s[0], *s[1:]), d) for s, d in zero_shapes]
        out_arrs = sharded(*concat_in, *zeros)
        return [
            {
                n: np.asarray(out_arrs[i]).reshape(
                    NCORES, *out_avals[i].shape)[c]
                for i, n in enumerate(out_names)
            }
            for c in range(NCORES)
        ]

    runner = {
        "run": run,
        "sharded": sharded,
        "in_names": in_names,
        "out_names": out_names,
        "out_avals": out_avals,
        "zero_shapes": zero_shapes,
        "mesh": mesh,
    }
    _NC_CACHE["runner"] = runner
    return runner


def kernel(**inputs) -> np.ndarray:
    in_maps = _prep_host(**inputs)
    results = get_runner()["run"](in_maps)
    out = np.empty((B, S, D), np.float32)
    for c in range(NCORES):
        b, hf = divmod(c, 2)
        out[b, hf * SQ:(hf + 1) * SQ, :] = results[c]["out_t"].T
    return out
